# revision 11
# baseline (speedup 1.0000x reference)
"""LoRA-MoE fused kernel for 8x Trainium2 NeuronCores (Bass/Tile).

Math (per batch sample b, data-parallel across 8 cores):
    g_b    = gate_w @ mean_s(x_b) + gate_b                      # [E]
    out_b  = x_b @ W^T + ((x_b @ A^T) * g_rep) @ Bt + bias      # [S, D_OUT]
where A = lora_A reshaped [E*R, D_IN], Bt[(e,r), o] = lora_B[e, o, r],
g_rep[(e,r)] = g_b[e].  The merged per-sample weights of the reference
(W + sum_e g[b,e] * lora_B[e] @ lora_A[e]) are never materialized.

v3: bf16 operands (FWL weight loads -> 216 ns/MM cadence at N=512;
rel err ~2.6e-3 vs the 2e-2 budget), fp32 PSUM.  Startup: junk warm-up
matmuls interleaved into the DMA-gated first chunk keep the PE's HAM
busy-window alive so the clock un-throttles ~10.8us in; inputs are
striped over three DMA rings (sync/scalar HWDGE + gpsimd SWDGE) with
only x + wt0/wt1 in the first ~14us.  Two deferred o_tiles cover the
x-load window base-only; their lora groups are wedged between steady
o_tiles so their vector-only PSUM drains hide under full tiles.  The
last o_tile runs sc-major with per-sc stores to shrink the tail.
"""

import sys

import numpy as np

try:
    import concourse.bass  # noqa: F401
except ImportError:  # pragma: no cover - fallback for bare environments
    for _p in (
        "/root/.axon_site",
        "/root/.axon_site/_ro/trn_rl_repo",
        "/root/.axon_site/_ro/pypackages",
        "/opt/trn_rl_repo",
    ):
        if _p not in sys.path:
            sys.path.append(_p)

import ml_dtypes
import concourse.bass as bass  # noqa: F401
import concourse.mybir as mybir
import concourse.tile as tile
from concourse import bacc, bass_utils

S, B, D_IN, D_OUT, E, R = 2048, 8, 1024, 4096, 8, 16
NCORES = 8
ER = E * R            # 128 (one partition dim worth of lora rows)
KC = D_IN // 128      # 8 contraction chunks
NOT = D_OUT // 128    # 32 output tiles
SC = 512              # s-chunk (one PSUM bank of f32)
NSC = S // SC         # 4
NDEFER = 2            # leading o_tiles processed base-only; lora added later

F32 = mybir.dt.float32
BF16 = mybir.dt.bfloat16
NPBF16 = ml_dtypes.bfloat16

Ident = mybir.ActivationFunctionType.Identity
CopyF = mybir.ActivationFunctionType.Copy


def _build_nc(n_cores: int = NCORES):
    nc = bacc.Bacc(
        "TRN2", target_bir_lowering=False, debug=False, num_devices=n_cores
    )

    xT = nc.dram_tensor("xT", [D_IN, S], BF16, kind="ExternalInput").ap()
    WTb = nc.dram_tensor("WTb", [NOT, 128, D_IN], BF16, kind="ExternalInput").ap()
    AT = nc.dram_tensor("AT", [128, KC, ER], BF16, kind="ExternalInput").ap()
    Bt = nc.dram_tensor("Bt", [ER, D_OUT], BF16, kind="ExternalInput").ap()
    gwT = nc.dram_tensor("gwT", [128, KC, ER], F32, kind="ExternalInput").ap()
    gb = nc.dram_tensor("gb", [ER, 1], F32, kind="ExternalInput").ap()
    bias_t = nc.dram_tensor("bias_t", [128, NOT], F32, kind="ExternalInput").ap()
    outT = nc.dram_tensor("outT", [D_OUT, S], BF16, kind="ExternalOutput").ap()

    with (
        tile.TileContext(nc) as tc,
        tc.tile_pool(name="singles", bufs=1) as singles,
        tc.tile_pool(name="wpool", bufs=6) as wpool,
        tc.tile_pool(name="opool", bufs=3) as opool,
        tc.tile_pool(name="odefer", bufs=NDEFER) as odefer,
        tc.tile_pool(name="ps_a", bufs=4, space="PSUM") as ps_a,
        tc.tile_pool(name="ps_b", bufs=4, space="PSUM") as ps_b,
    ):
        # ---- junk warm-up operand tiles (memset so CoreSim sees them
        # initialized); the junk matmuls themselves are interleaved into
        # the DMA-gated start below.
        jw = singles.tile([128, 128], BF16)
        jx = singles.tile([128, SC], BF16)
        nc.gpsimd.memset(jw[:], 0.0)
        nc.gpsimd.memset(jx[:], 0.0)
        jp = ps_b.tile([128, SC], F32, tag="acc")

        def junk(n):
            for _ in range(n):
                nc.tensor.matmul(jp[:], jw[:], jx[:], start=True, stop=True)

        # ---- scalar HWDGE ring: wt0, wt1, x0 q2/q3, x2, x5, AT, gw,
        # wt2..wt5.  Ring order == program order of dma_start calls.
        _wt_cache = {}

        def wt_load(ot):
            if ot in _wt_cache:
                return _wt_cache.pop(ot)
            wt = wpool.tile([128, KC, 128], BF16, tag="wt")
            nc.scalar.dma_start(
                out=wt[:], in_=WTb[ot].rearrange("p (c m) -> p c m", c=KC)
            )
            return wt

        def wt_prefetch(ot):
            wt = wpool.tile([128, KC, 128], BF16, tag="wt")
            nc.scalar.dma_start(
                out=wt[:], in_=WTb[ot].rearrange("p (c m) -> p c m", c=KC)
            )
            _wt_cache[ot] = wt

        x_sb = singles.tile([128, KC, S], BF16)
        x_r = xT.rearrange("(c p) s -> c p s", p=128)

        def x_dma(eng, c):
            eng.dma_start(out=x_sb[:, c, :], in_=x_r[c])

        wt_prefetch(0)
        wt_prefetch(1)
        for q in (2, 3):
            nc.scalar.dma_start(
                out=x_sb[:, 0, q * SC : (q + 1) * SC],
                in_=x_r[0][:, q * SC : (q + 1) * SC],
            )
        x_dma(nc.scalar, 2)
        x_dma(nc.scalar, 5)
        gw_sb = singles.tile([128, KC, ER], F32)
        nc.scalar.dma_start(out=gw_sb[:], in_=gwT[:])
        wt_prefetch(2)
        wt_prefetch(3)
        wt_prefetch(4)
        wt_prefetch(5)

        # ---- sync HWDGE ring: x0 q0/q1, x1, x4, x7, bias, gb, Bt.
        for q in (0, 1):
            nc.sync.dma_start(
                out=x_sb[:, 0, q * SC : (q + 1) * SC],
                in_=x_r[0][:, q * SC : (q + 1) * SC],
            )
        x_dma(nc.sync, 1)
        x_dma(nc.sync, 4)
        x_dma(nc.sync, 7)
        bias_sb = singles.tile([128, NOT], F32)
        nc.sync.dma_start(out=bias_sb[:], in_=bias_t)
        gb_sb = singles.tile([128, 1], F32)
        nc.sync.dma_start(out=gb_sb[:], in_=gb)
        bt_sb = singles.tile([128, D_OUT], BF16)
        nc.sync.dma_start(out=bt_sb[:], in_=Bt)

        # ---- gpsimd SWDGE ring: AT first (u matmuls run inside phase A),
        # then x3, x6 (third bandwidth channel).
        at_sb = singles.tile([128, KC, ER], BF16)
        nc.gpsimd.dma_start(out=at_sb[:], in_=AT[:])
        x_dma(nc.gpsimd, 3)
        x_dma(nc.gpsimd, 6)

        _ps_toggle = [0]

        def psum_group():
            pool = ps_a if _ps_toggle[0] % 2 == 0 else ps_b
            _ps_toggle[0] += 1
            return [
                pool.tile([128, SC], F32, tag="acc", name="acc")
                for _ in range(NSC)
            ]

        def xs(c, sc):
            return x_sb[:, c, sc * SC : (sc + 1) * SC]

        def us(sc):
            return u_sb[:, sc * SC : (sc + 1) * SC]

        def bias_copy(o_sb, accs, ot, scs=range(NSC)):
            for sc in scs:
                sl = slice(sc * SC, (sc + 1) * SC)
                if (ot + sc) % 2 == 0:
                    nc.vector.tensor_scalar_add(
                        o_sb[:, sl], accs[sc][:], bias_sb[:, ot : ot + 1]
                    )
                else:
                    nc.scalar.activation(
                        out=o_sb[:, sl],
                        in_=accs[sc][:],
                        func=Ident,
                        bias=bias_sb[:, ot : ot + 1],
                        scale=1.0,
                    )

        defer_wt = [wt_load(0), wt_load(1)]
        defer_o = [
            odefer.tile([128, S], F32, tag="od", name="od") for _ in range(NDEFER)
        ]
        u_sb = singles.tile([128, S], BF16)

        # ---- Pass 1: ot0/ot1 over sc0-sc1 PLUS all four u s-chunks, all
        # 8 contraction chunks, c-major: 12 matmuls per arriving x chunk
        # makes the DMA-bound window PE-bound, so the PE has no idle gaps
        # and the HAM clock un-throttles ~3.4us after the first junk MM.
        # PSUM: ot halves on ps_a (4 banks), u group on ps_b (4 banks).
        pA = psum_group()   # [ot0-sc0, ot0-sc1, ot1-sc0, ot1-sc1] on ps_a
        ug = psum_group()   # u sc0-3 on ps_b (sc3 aliases jp's bank)
        junk(5)
        nc.tensor.matmul(pA[0][:], defer_wt[0][:, 0, :], xs(0, 0),
                         start=True, stop=False)
        nc.tensor.matmul(pA[1][:], defer_wt[0][:, 0, :], xs(0, 1),
                         start=True, stop=False)
        nc.tensor.matmul(pA[2][:], defer_wt[1][:, 0, :], xs(0, 0),
                         start=True, stop=False)
        nc.tensor.matmul(pA[3][:], defer_wt[1][:, 0, :], xs(0, 1),
                         start=True, stop=False)
        nc.tensor.matmul(ug[0][:], at_sb[:, 0, :], xs(0, 0),
                         start=True, stop=False)
        nc.tensor.matmul(ug[1][:], at_sb[:, 0, :], xs(0, 1),
                         start=True, stop=False)
        junk(2)
        nc.tensor.matmul(ug[2][:], at_sb[:, 0, :], xs(0, 2),
                         start=True, stop=False)
        junk(2)
        # u-sc3's accumulation is rotated to start at c4 (chunk order
        # c4..c7, c0..c3): its bank -- the junk tile's alias -- then has
        # its first real write only ~21us in, so junk matmuls stay legal
        # between the chunk blocks below and keep the HAM busy-window
        # alive across late x-chunk arrivals.
        for c in range(1, KC):
            last = c == KC - 1
            for ot in (0, 1):
                for h in (0, 1):
                    nc.tensor.matmul(
                        pA[2 * ot + h][:], defer_wt[ot][:, c, :], xs(c, h),
                        start=False, stop=last,
                    )
                if last:
                    bias_copy(
                        defer_o[ot],
                        {0: pA[2 * ot], 1: pA[2 * ot + 1]}, ot, scs=(0, 1),
                    )
            for sc in range(3):
                nc.tensor.matmul(
                    ug[sc][:], at_sb[:, c, :], xs(c, sc),
                    start=False, stop=last,
                )
            if c >= 4:
                nc.tensor.matmul(
                    ug[3][:], at_sb[:, c, :], xs(c, 3),
                    start=(c == 4), stop=False,
                )
            if c == 1:
                junk(2)
            elif c in (2, 3):
                junk(3)
        for c in range(4):
            nc.tensor.matmul(
                ug[3][:], at_sb[:, c, :], xs(c, 3),
                start=False, stop=(c == 3),
            )
        for sc in range(NSC):
            nc.vector.tensor_copy(us(sc), ug[sc][:])

        # ---- per-chunk column sums for the gate (vector/scalar split)
        xsum = singles.tile([128, KC], F32)
        scratch = singles.tile([128, S], BF16)
        for c in range(KC):
            if c % 2 == 0:
                nc.vector.reduce_sum(
                    out=xsum[:, c : c + 1], in_=x_sb[:, c, :],
                    axis=mybir.AxisListType.X,
                )
            else:
                nc.scalar.activation(
                    out=scratch[:], in_=x_sb[:, c, :],
                    func=CopyF, accum_out=xsum[:, c : c + 1],
                )

        # ---- Pass 2: ot0/ot1 over sc2-sc3, dense from SBUF; banks reuse
        # pass 1's ps_a slots (their drains were queued at c7 per-ot).
        pB = psum_group()   # [ot0-sc2, ot0-sc3, ot1-sc2, ot1-sc3]
        for c in range(KC):
            last = c == KC - 1
            for ot in (0, 1):
                for h in (0, 1):
                    nc.tensor.matmul(
                        pB[2 * ot + h][:], defer_wt[ot][:, c, :], xs(c, 2 + h),
                        start=(c == 0), stop=last,
                    )
                if last:
                    bias_copy(
                        defer_o[ot],
                        {2: pB[2 * ot], 3: pB[2 * ot + 1]}, ot, scs=(2, 3),
                    )

        # ---- gate: g128[er] = sum_c gw_sb[:,c,:]^T @ xsum[:,c] + gb
        # (gwT pre-scaled by 1/S on the host, so xsum acts as the mean).
        # g_ps reuses u-sc0's just-drained bank.
        g_ps = ps_b.tile([128, 1], F32, tag="acc")
        for c in range(KC):
            nc.tensor.matmul(
                g_ps[:], gw_sb[:, c, :], xsum[:, c : c + 1],
                start=(c == 0), stop=(c == KC - 1),
            )
        g_sb = singles.tile([128, 1], F32)
        nc.vector.tensor_add(g_sb[:], g_ps[:], gb_sb[:])

        # fold the gate into Bt: bts[er, o] = g[er] * Bt[er, o],
        # split across Vector and Scalar engines
        bts_sb = singles.tile([128, D_OUT], BF16)
        half = D_OUT // 2
        nc.vector.tensor_scalar_mul(bts_sb[:, :half], bt_sb[:, :half], g_sb[:])
        nc.scalar.activation(
            out=bts_sb[:, half:], in_=bt_sb[:, half:], func=Ident, scale=g_sb[:]
        )

        def steady(ot, wt):
            o_sb = opool.tile([128, S], BF16, tag="o", name="o_sb")
            osl = slice(ot * 128, (ot + 1) * 128)
            accs = psum_group()
            # lora first: the bts weight load sits at the group boundary
            # instead of between the last base chunk and the drain
            for sc in range(NSC):
                nc.tensor.matmul(
                    accs[sc][:], bts_sb[:, osl], us(sc),
                    start=True, stop=False,
                )
            for c in range(KC):
                for sc in range(NSC):
                    nc.tensor.matmul(
                        accs[sc][:], wt[:, c, :], xs(c, sc),
                        start=False, stop=(c == KC - 1),
                    )
            bias_copy(o_sb, accs, ot)
            nc.sync.dma_start(out=outT[osl, :], in_=o_sb[:])

        def defer_lora(ot):
            osl = slice(ot * 128, (ot + 1) * 128)
            laccs = psum_group()
            for sc in range(NSC):
                nc.tensor.matmul(
                    laccs[sc][:], bts_sb[:, osl], us(sc),
                    start=True, stop=True,
                )
            st = opool.tile([128, S], BF16, tag="o", name="o_sb")
            for sc in range(NSC):
                sl = slice(sc * SC, (sc + 1) * SC)
                nc.vector.tensor_add(st[:, sl], defer_o[ot][:, sl], laccs[sc][:])
            nc.sync.dma_start(out=outT[osl, :], in_=st[:])

        # ---- steady loop; the two deferred-lora groups are wedged after
        # ot2/ot3 so their vector-only drains hide under full o_tiles.
        steady(2, wt_load(2))
        defer_lora(0)
        steady(3, wt_load(3))
        defer_lora(1)
        for ot in range(4, NOT - 1):
            steady(ot, wt_load(ot))

        # ---- last o_tile: sc-major so sc0-2's copies + stores run ~6us
        # before the end; only sc3's copy/store chain trails the last MM.
        ot = NOT - 1
        wt = wt_load(ot)
        o_sb = opool.tile([128, S], BF16, tag="o", name="o_sb")
        osl = slice(ot * 128, (ot + 1) * 128)
        accs = psum_group()
        for sc in range(NSC):
            nc.tensor.matmul(
                accs[sc][:], bts_sb[:, osl], us(sc),
                start=True, stop=False,
            )
            for c in range(KC):
                nc.tensor.matmul(
                    accs[sc][:], wt[:, c, :], xs(c, sc),
                    start=False, stop=(c == KC - 1),
                )
            sl = slice(sc * SC, (sc + 1) * SC)
            if sc < NSC - 1:
                bias_copy(o_sb, accs, ot, scs=(sc,))
                nc.sync.dma_start(out=outT[osl, sl], in_=o_sb[:, sl])
            else:
                # final copy all on vector (it picks up the stop-MM sem
                # faster than scalar in practice), then one store
                nc.vector.tensor_scalar_add(
                    o_sb[:, sl], accs[sc][:], bias_sb[:, ot : ot + 1]
                )
                nc.sync.dma_start(out=outT[osl, sl], in_=o_sb[:, sl])

    nc.compile()
    return nc


def _prep_in_maps(x, gate_w, gate_b, W, bias, lora_A, lora_B):
    f32 = np.float32
    x = np.asarray(x, f32)
    gate_w = np.asarray(gate_w, f32)
    gate_b = np.asarray(gate_b, f32)
    W = np.asarray(W, f32)
    bias = np.asarray(bias, f32)
    lora_A = np.asarray(lora_A, f32)
    lora_B = np.asarray(lora_B, f32)

    WTb = np.ascontiguousarray(
        W.reshape(NOT, 128, KC, 128).transpose(0, 3, 2, 1).reshape(NOT, 128, D_IN)
    ).astype(NPBF16)
    AT = np.ascontiguousarray(
        lora_A.reshape(ER, D_IN).T.reshape(KC, 128, ER).transpose(1, 0, 2)
    ).astype(NPBF16)
    Bt = np.ascontiguousarray(lora_B.transpose(0, 2, 1).reshape(ER, D_OUT)).astype(
        NPBF16
    )
    gwT = np.ascontiguousarray(
        (np.repeat(gate_w, R, axis=0).T / np.float32(S))
        .reshape(KC, 128, ER)
        .transpose(1, 0, 2)
    ).astype(f32)
    gbr = np.ascontiguousarray(np.repeat(gate_b, R).reshape(ER, 1))
    bias_t = np.ascontiguousarray(bias.reshape(NOT, 128).T)

    shared = {
        "WTb": WTb,
        "AT": AT,
        "Bt": Bt,
        "gwT": gwT,
        "gb": gbr,
        "bias_t": bias_t,
    }
    in_maps = []
    for b in range(NCORES):
        m = dict(shared)
        m["xT"] = np.ascontiguousarray(x[:, b, :].T).astype(NPBF16)
        in_maps.append(m)
    return in_maps


def run(inputs, trace=False, trace_cores=None):
    """Build + run on 8 cores. Returns (out [S,B,D_OUT], BassKernelResults)."""
    in_maps = _prep_in_maps(**inputs)
    nc = _build_nc()
    kwargs = {}
    if trace:
        _register_axon_ntff_hook()
        kwargs = dict(trace=True, trace_cores=trace_cores or [0])
    res = bass_utils.run_bass_kernel_spmd(
        nc, in_maps, core_ids=list(range(NCORES)), **kwargs
    )
    out = np.empty((S, B, D_OUT), np.float32)
    for b in range(NCORES):
        out[:, b, :] = np.asarray(res.results[b]["outT"]).astype(np.float32).T
    return out, res


def _register_axon_ntff_hook():
    """antenv.axon_hooks is missing on this image; synthesize it so
    run_bass_kernel_spmd(trace=True) can reach the axon NTFF profiler."""
    import types

    try:
        from antenv.axon_hooks import get_axon_ntff_profile_hook  # noqa: F401

        return  # real module present
    except ImportError:
        pass
    try:
        from trn_agent_boot.trn_boot import _ntff_profile_via_ctypes
    except ImportError:
        return
    import antenv

    mod = types.ModuleType("antenv.axon_hooks")
    _state = {"hook": None}
    mod.set_axon_ntff_profile_hook = lambda h: _state.__setitem__("hook", h)
    mod.get_axon_ntff_profile_hook = lambda: _state["hook"]
    sys.modules["antenv.axon_hooks"] = mod
    antenv.axon_hooks = mod
    hook = _ntff_profile_via_ctypes("/opt/axon/libaxon_pjrt.so")
    if hook is not None:
        mod.set_axon_ntff_profile_hook(hook)


def kernel(**inputs) -> np.ndarray:
    out, _ = run(inputs, trace=False)
    return out


# revision 17
# speedup vs baseline: 1.0052x; 1.0052x over previous
"""LoRA-MoE fused kernel for 8x Trainium2 NeuronCores (Bass/Tile).

Math (per batch sample b, data-parallel across 8 cores):
    g_b    = gate_w @ mean_s(x_b) + gate_b                      # [E]
    out_b  = x_b @ W^T + ((x_b @ A^T) * g_rep) @ Bt + bias      # [S, D_OUT]
where A = lora_A reshaped [E*R, D_IN], Bt[(e,r), o] = lora_B[e, o, r],
g_rep[(e,r)] = g_b[e].  The merged per-sample weights of the reference
(W + sum_e g[b,e] * lora_B[e] @ lora_A[e]) are never materialized.

v3: bf16 operands (FWL weight loads -> 216 ns/MM cadence at N=512;
rel err ~2.6e-3 vs the 2e-2 budget), fp32 PSUM.  Startup: junk warm-up
matmuls interleaved into the DMA-gated first chunk keep the PE's HAM
busy-window alive so the clock un-throttles ~10.8us in; inputs are
striped over three DMA rings (sync/scalar HWDGE + gpsimd SWDGE) with
only x + wt0/wt1 in the first ~14us.  Two deferred o_tiles cover the
x-load window base-only; their lora groups are wedged between steady
o_tiles so their vector-only PSUM drains hide under full tiles.  The
last o_tile runs sc-major with per-sc stores to shrink the tail.
"""

import sys

import numpy as np

try:
    import concourse.bass  # noqa: F401
except ImportError:  # pragma: no cover - fallback for bare environments
    for _p in (
        "/root/.axon_site",
        "/root/.axon_site/_ro/trn_rl_repo",
        "/root/.axon_site/_ro/pypackages",
        "/opt/trn_rl_repo",
    ):
        if _p not in sys.path:
            sys.path.append(_p)

import ml_dtypes
import concourse.bass as bass  # noqa: F401
import concourse.mybir as mybir
import concourse.tile as tile
from concourse import bacc, bass_utils

S, B, D_IN, D_OUT, E, R = 2048, 8, 1024, 4096, 8, 16
NCORES = 8
ER = E * R            # 128 (one partition dim worth of lora rows)
KC = D_IN // 128      # 8 contraction chunks
NOT = D_OUT // 128    # 32 output tiles
SC = 512              # s-chunk (one PSUM bank of f32)
NSC = S // SC         # 4
NDEFER = 2            # leading o_tiles processed base-only; lora added later

F32 = mybir.dt.float32
BF16 = mybir.dt.bfloat16
NPBF16 = ml_dtypes.bfloat16

Ident = mybir.ActivationFunctionType.Identity
CopyF = mybir.ActivationFunctionType.Copy


def _build_nc(n_cores: int = NCORES):
    nc = bacc.Bacc(
        "TRN2", target_bir_lowering=False, debug=False, num_devices=n_cores
    )

    xT = nc.dram_tensor("xT", [D_IN, S], BF16, kind="ExternalInput").ap()
    WTb = nc.dram_tensor("WTb", [NOT, 128, D_IN], BF16, kind="ExternalInput").ap()
    AT = nc.dram_tensor("AT", [128, KC, ER], BF16, kind="ExternalInput").ap()
    Bt = nc.dram_tensor("Bt", [ER, D_OUT], BF16, kind="ExternalInput").ap()
    gwT = nc.dram_tensor("gwT", [128, KC, ER], F32, kind="ExternalInput").ap()
    gb = nc.dram_tensor("gb", [ER, 1], F32, kind="ExternalInput").ap()
    bias_t = nc.dram_tensor("bias_t", [128, NOT], F32, kind="ExternalInput").ap()
    outT = nc.dram_tensor("outT", [D_OUT, S], BF16, kind="ExternalOutput").ap()

    with (
        tile.TileContext(nc) as tc,
        tc.tile_pool(name="singles", bufs=1) as singles,
        tc.tile_pool(name="wpool", bufs=6) as wpool,
        tc.tile_pool(name="opool", bufs=3) as opool,
        tc.tile_pool(name="odefer", bufs=NDEFER) as odefer,
        tc.tile_pool(name="ps_a", bufs=4, space="PSUM") as ps_a,
        tc.tile_pool(name="ps_b", bufs=4, space="PSUM") as ps_b,
    ):
        # ---- junk warm-up operand tiles (memset so CoreSim sees them
        # initialized); the junk matmuls themselves are interleaved into
        # the DMA-gated start below.
        jw = singles.tile([128, 128], BF16)
        jx = singles.tile([128, SC], BF16)
        nc.gpsimd.memset(jw[:], 0.0)
        nc.gpsimd.memset(jx[:], 0.0)
        jp = ps_b.tile([128, SC], F32, tag="acc")

        def junk(n):
            for _ in range(n):
                nc.tensor.matmul(jp[:], jw[:], jx[:], start=True, stop=True)

        # ---- scalar HWDGE ring: wt0, wt1, x0 q2/q3, x2, x5, AT, gw,
        # wt2..wt5.  Ring order == program order of dma_start calls.
        _wt_cache = {}

        def wt_load(ot):
            if ot in _wt_cache:
                return _wt_cache.pop(ot)
            wt = wpool.tile([128, KC, 128], BF16, tag="wt")
            nc.scalar.dma_start(
                out=wt[:], in_=WTb[ot].rearrange("p (c m) -> p c m", c=KC)
            )
            return wt

        def wt_prefetch(ot):
            wt = wpool.tile([128, KC, 128], BF16, tag="wt")
            nc.scalar.dma_start(
                out=wt[:], in_=WTb[ot].rearrange("p (c m) -> p c m", c=KC)
            )
            _wt_cache[ot] = wt

        x_sb = singles.tile([128, KC, S], BF16)
        x_r = xT.rearrange("(c p) s -> c p s", p=128)

        def x_dma(eng, c):
            eng.dma_start(out=x_sb[:, c, :], in_=x_r[c])

        wt_prefetch(0)
        wt_prefetch(1)
        for q in (2, 3):
            nc.scalar.dma_start(
                out=x_sb[:, 0, q * SC : (q + 1) * SC],
                in_=x_r[0][:, q * SC : (q + 1) * SC],
            )
        x_dma(nc.scalar, 5)
        x_dma(nc.scalar, 7)
        gw_sb = singles.tile([128, KC, ER], F32)
        nc.scalar.dma_start(out=gw_sb[:], in_=gwT[:])
        wt_prefetch(2)
        wt_prefetch(3)
        wt_prefetch(4)
        wt_prefetch(5)

        # ---- sync HWDGE ring: x0 q0/q1, x1, x4, x6, bias, gb, Bt.
        for q in (0, 1):
            nc.sync.dma_start(
                out=x_sb[:, 0, q * SC : (q + 1) * SC],
                in_=x_r[0][:, q * SC : (q + 1) * SC],
            )
        x_dma(nc.sync, 1)
        x_dma(nc.sync, 4)
        x_dma(nc.sync, 6)
        bias_sb = singles.tile([128, NOT], F32)
        nc.sync.dma_start(out=bias_sb[:], in_=bias_t)
        gb_sb = singles.tile([128, 1], F32)
        nc.sync.dma_start(out=gb_sb[:], in_=gb)
        bt_sb = singles.tile([128, D_OUT], BF16)
        nc.sync.dma_start(out=bt_sb[:], in_=Bt)

        # ---- gpsimd SWDGE ring: AT first (u matmuls run inside phase A),
        # then x2, x3 (third bandwidth channel; the scalar ring is the
        # slowest early since it carries wt0/wt1 + the x0 quarters).
        at_sb = singles.tile([128, KC, ER], BF16)
        nc.gpsimd.dma_start(out=at_sb[:], in_=AT[:])
        x_dma(nc.gpsimd, 2)
        x_dma(nc.gpsimd, 3)

        def psum_group(pool):
            return [
                pool.tile([128, SC], F32, tag="acc", name="acc")
                for _ in range(NSC)
            ]

        def xs(c, sc):
            return x_sb[:, c, sc * SC : (sc + 1) * SC]

        def us(sc):
            return u_sb[:, sc * SC : (sc + 1) * SC]

        def bias_copy(o_sb, accs, ot, scs=range(NSC)):
            for sc in scs:
                sl = slice(sc * SC, (sc + 1) * SC)
                if (ot + sc) % 2 == 0:
                    nc.vector.tensor_scalar_add(
                        o_sb[:, sl], accs[sc][:], bias_sb[:, ot : ot + 1]
                    )
                else:
                    nc.scalar.activation(
                        out=o_sb[:, sl],
                        in_=accs[sc][:],
                        func=Ident,
                        bias=bias_sb[:, ot : ot + 1],
                        scale=1.0,
                    )

        defer_wt = [wt_load(0), wt_load(1)]
        defer_o = [
            odefer.tile([128, S], F32, tag="od", name="od") for _ in range(NDEFER)
        ]
        u_sb = singles.tile([128, S], BF16)

        # ---- Pass 1: ot0/ot1 over sc0-sc1 PLUS all four u s-chunks, all
        # 8 contraction chunks, c-major: 12 matmuls per arriving x chunk
        # makes the DMA-bound window PE-bound, so the PE has no idle gaps
        # and the HAM clock un-throttles ~3.4us after the first junk MM.
        # PSUM: ot halves on ps_a (4 banks), u group on ps_b (4 banks).
        pA = psum_group(ps_a)  # [ot0-sc0, ot0-sc1, ot1-sc0, ot1-sc1]
        ug = psum_group(ps_b)  # u sc0-3 (sc3 aliases jp's bank)
        junk(5)
        nc.tensor.matmul(pA[0][:], defer_wt[0][:, 0, :], xs(0, 0),
                         start=True, stop=False)
        nc.tensor.matmul(pA[1][:], defer_wt[0][:, 0, :], xs(0, 1),
                         start=True, stop=False)
        nc.tensor.matmul(pA[2][:], defer_wt[1][:, 0, :], xs(0, 0),
                         start=True, stop=False)
        nc.tensor.matmul(pA[3][:], defer_wt[1][:, 0, :], xs(0, 1),
                         start=True, stop=False)
        nc.tensor.matmul(ug[0][:], at_sb[:, 0, :], xs(0, 0),
                         start=True, stop=False)
        nc.tensor.matmul(ug[1][:], at_sb[:, 0, :], xs(0, 1),
                         start=True, stop=False)
        junk(2)
        nc.tensor.matmul(ug[2][:], at_sb[:, 0, :], xs(0, 2),
                         start=True, stop=False)
        junk(2)
        # u-sc3's accumulation is rotated to start at c4 (chunk order
        # c4..c7, c0..c3): its bank -- the junk tile's alias -- then has
        # its first real write only ~21us in, so junk matmuls stay legal
        # between the chunk blocks below and keep the HAM busy-window
        # alive across late x-chunk arrivals.
        for c in range(1, KC):
            last = c == KC - 1
            for ot in (0, 1):
                for h in (0, 1):
                    nc.tensor.matmul(
                        pA[2 * ot + h][:], defer_wt[ot][:, c, :], xs(c, h),
                        start=False, stop=last,
                    )
                if last:
                    bias_copy(
                        defer_o[ot],
                        {0: pA[2 * ot], 1: pA[2 * ot + 1]}, ot, scs=(0, 1),
                    )
            for sc in range(3):
                nc.tensor.matmul(
                    ug[sc][:], at_sb[:, c, :], xs(c, sc),
                    start=False, stop=last,
                )
            if c >= 4:
                nc.tensor.matmul(
                    ug[3][:], at_sb[:, c, :], xs(c, 3),
                    start=(c == 4), stop=False,
                )
            if c == 1:
                junk(2)
            elif c in (2, 3):
                junk(3)
        for c in range(4):
            nc.tensor.matmul(
                ug[3][:], at_sb[:, c, :], xs(c, 3),
                start=False, stop=(c == 3),
            )
        for sc in range(NSC):
            nc.vector.tensor_copy(us(sc), ug[sc][:])

        # ---- per-chunk column sums for the gate (vector/scalar split)
        xsum = singles.tile([128, KC], F32)
        scratch = singles.tile([128, S], BF16)
        for c in range(KC):
            if c % 2 == 0:
                nc.vector.reduce_sum(
                    out=xsum[:, c : c + 1], in_=x_sb[:, c, :],
                    axis=mybir.AxisListType.X,
                )
            else:
                nc.scalar.activation(
                    out=scratch[:], in_=x_sb[:, c, :],
                    func=CopyF, accum_out=xsum[:, c : c + 1],
                )

        # ---- Pass 2: ot0/ot1 over sc2-sc3, dense from SBUF; banks reuse
        # pass 1's ps_a slots (their drains were queued at c7 per-ot).
        pB = psum_group(ps_a)  # [ot0-sc2, ot0-sc3, ot1-sc2, ot1-sc3]
        for c in range(KC):
            last = c == KC - 1
            for ot in (0, 1):
                for h in (0, 1):
                    nc.tensor.matmul(
                        pB[2 * ot + h][:], defer_wt[ot][:, c, :], xs(c, 2 + h),
                        start=(c == 0), stop=last,
                    )
                if last:
                    bias_copy(
                        defer_o[ot],
                        {2: pB[2 * ot], 3: pB[2 * ot + 1]}, ot, scs=(2, 3),
                    )

        # ---- gate: g128[er] = sum_c gw_sb[:,c,:]^T @ xsum[:,c] + gb
        # (gwT pre-scaled by 1/S on the host, so xsum acts as the mean).
        # g_ps reuses u-sc0's just-drained bank.
        g_ps = ps_b.tile([128, 1], F32, tag="acc")
        for c in range(KC):
            nc.tensor.matmul(
                g_ps[:], gw_sb[:, c, :], xsum[:, c : c + 1],
                start=(c == 0), stop=(c == KC - 1),
            )
        g_sb = singles.tile([128, 1], F32)
        nc.vector.tensor_add(g_sb[:], g_ps[:], gb_sb[:])

        # fold the gate into Bt: bts[er, o] = g[er] * Bt[er, o],
        # split across Vector and Scalar engines
        bts_sb = singles.tile([128, D_OUT], BF16)
        half = D_OUT // 2
        nc.vector.tensor_scalar_mul(bts_sb[:, :half], bt_sb[:, :half], g_sb[:])
        nc.scalar.activation(
            out=bts_sb[:, half:], in_=bt_sb[:, half:], func=Ident, scale=g_sb[:]
        )

        def steady(ot, wt, wedge=None):
            o_sb = opool.tile([128, S], BF16, tag="o", name="o_sb")
            osl = slice(ot * 128, (ot + 1) * 128)
            # pools alternate by ot parity; the deferred-lora wedges always
            # use ps_a inside an even (ps_b) tile, so every group gets a
            # full tile's worth of separation before its banks are reused.
            accs = psum_group(ps_b if ot % 2 == 0 else ps_a)
            for sc in range(NSC):
                nc.tensor.matmul(
                    accs[sc][:], bts_sb[:, osl], us(sc),
                    start=True, stop=False,
                )
            for c in range(KC):
                if c == 4 and wedge is not None:
                    wedge()
                for sc in range(NSC):
                    nc.tensor.matmul(
                        accs[sc][:], wt[:, c, :], xs(c, sc),
                        start=False, stop=(c == KC - 1),
                    )
            bias_copy(o_sb, accs, ot)
            nc.sync.dma_start(out=outT[osl, :], in_=o_sb[:])

        def defer_lora(ot):
            osl = slice(ot * 128, (ot + 1) * 128)
            laccs = psum_group(ps_a)
            for sc in range(NSC):
                nc.tensor.matmul(
                    laccs[sc][:], bts_sb[:, osl], us(sc),
                    start=True, stop=True,
                )
            st = opool.tile([128, S], BF16, tag="o", name="o_sb")
            for sc in range(NSC):
                sl = slice(sc * SC, (sc + 1) * SC)
                nc.vector.tensor_add(st[:, sl], defer_o[ot][:, sl], laccs[sc][:])
            nc.sync.dma_start(out=outT[osl, :], in_=st[:])

        # ---- steady loop; the two deferred-lora groups are wedged mid-
        # stream into ot2/ot4 so their vector-only drains hide under the
        # enclosing tile's second half.
        steady(2, wt_load(2), wedge=lambda: defer_lora(0))
        steady(3, wt_load(3))
        steady(4, wt_load(4), wedge=lambda: defer_lora(1))
        for ot in range(5, NOT - 1):
            steady(ot, wt_load(ot))

        # ---- last o_tile: sc-major so sc0-2's copies + stores run ~6us
        # before the end; only sc3's copy/store chain trails the last MM.
        ot = NOT - 1
        wt = wt_load(ot)
        o_sb = opool.tile([128, S], BF16, tag="o", name="o_sb")
        osl = slice(ot * 128, (ot + 1) * 128)
        accs = psum_group(ps_b if ot % 2 == 0 else ps_a)
        for sc in range(NSC):
            nc.tensor.matmul(
                accs[sc][:], bts_sb[:, osl], us(sc),
                start=True, stop=False,
            )
            for c in range(KC):
                nc.tensor.matmul(
                    accs[sc][:], wt[:, c, :], xs(c, sc),
                    start=False, stop=(c == KC - 1),
                )
            sl = slice(sc * SC, (sc + 1) * SC)
            if sc < NSC - 1:
                bias_copy(o_sb, accs, ot, scs=(sc,))
                nc.sync.dma_start(out=outT[osl, sl], in_=o_sb[:, sl])
            else:
                # final copy all on vector (it picks up the stop-MM sem
                # faster than scalar in practice), then one store
                nc.vector.tensor_scalar_add(
                    o_sb[:, sl], accs[sc][:], bias_sb[:, ot : ot + 1]
                )
                nc.sync.dma_start(out=outT[osl, sl], in_=o_sb[:, sl])

    nc.compile()
    return nc


def _prep_in_maps(x, gate_w, gate_b, W, bias, lora_A, lora_B):
    f32 = np.float32
    x = np.asarray(x, f32)
    gate_w = np.asarray(gate_w, f32)
    gate_b = np.asarray(gate_b, f32)
    W = np.asarray(W, f32)
    bias = np.asarray(bias, f32)
    lora_A = np.asarray(lora_A, f32)
    lora_B = np.asarray(lora_B, f32)

    WTb = np.ascontiguousarray(
        W.reshape(NOT, 128, KC, 128).transpose(0, 3, 2, 1).reshape(NOT, 128, D_IN)
    ).astype(NPBF16)
    AT = np.ascontiguousarray(
        lora_A.reshape(ER, D_IN).T.reshape(KC, 128, ER).transpose(1, 0, 2)
    ).astype(NPBF16)
    Bt = np.ascontiguousarray(lora_B.transpose(0, 2, 1).reshape(ER, D_OUT)).astype(
        NPBF16
    )
    gwT = np.ascontiguousarray(
        (np.repeat(gate_w, R, axis=0).T / np.float32(S))
        .reshape(KC, 128, ER)
        .transpose(1, 0, 2)
    ).astype(f32)
    gbr = np.ascontiguousarray(np.repeat(gate_b, R).reshape(ER, 1))
    bias_t = np.ascontiguousarray(bias.reshape(NOT, 128).T)

    shared = {
        "WTb": WTb,
        "AT": AT,
        "Bt": Bt,
        "gwT": gwT,
        "gb": gbr,
        "bias_t": bias_t,
    }
    in_maps = []
    for b in range(NCORES):
        m = dict(shared)
        m["xT"] = np.ascontiguousarray(x[:, b, :].T).astype(NPBF16)
        in_maps.append(m)
    return in_maps


def run(inputs, trace=False, trace_cores=None):
    """Build + run on 8 cores. Returns (out [S,B,D_OUT], BassKernelResults)."""
    in_maps = _prep_in_maps(**inputs)
    nc = _build_nc()
    kwargs = {}
    if trace:
        _register_axon_ntff_hook()
        kwargs = dict(trace=True, trace_cores=trace_cores or [0])
    res = bass_utils.run_bass_kernel_spmd(
        nc, in_maps, core_ids=list(range(NCORES)), **kwargs
    )
    out = np.empty((S, B, D_OUT), np.float32)
    for b in range(NCORES):
        out[:, b, :] = np.asarray(res.results[b]["outT"]).astype(np.float32).T
    return out, res


def _register_axon_ntff_hook():
    """antenv.axon_hooks is missing on this image; synthesize it so
    run_bass_kernel_spmd(trace=True) can reach the axon NTFF profiler."""
    import types

    try:
        from antenv.axon_hooks import get_axon_ntff_profile_hook  # noqa: F401

        return  # real module present
    except ImportError:
        pass
    try:
        from trn_agent_boot.trn_boot import _ntff_profile_via_ctypes
    except ImportError:
        return
    import antenv

    mod = types.ModuleType("antenv.axon_hooks")
    _state = {"hook": None}
    mod.set_axon_ntff_profile_hook = lambda h: _state.__setitem__("hook", h)
    mod.get_axon_ntff_profile_hook = lambda: _state["hook"]
    sys.modules["antenv.axon_hooks"] = mod
    antenv.axon_hooks = mod
    hook = _ntff_profile_via_ctypes("/opt/axon/libaxon_pjrt.so")
    if hook is not None:
        mod.set_axon_ntff_profile_hook(hook)


def kernel(**inputs) -> np.ndarray:
    out, _ = run(inputs, trace=False)
    return out
